# revision 3
# baseline (speedup 1.0000x reference)
"""Trainium2 Bass kernel for KeyChannelwiseMemoryMultiHead.

Math: for each pixel vector x (256):
  y1 = w_in @ x + b_in                      (512 = 64 key x 8 heads, chan = k*8+n)
  a[n,m] = sum_k key_p[n,k,m] * y1[k*8+n]   (per-head key matmul)
  s = softmax_m(a[n,:])
  z[n,d] = sum_m memory[n,m,d] * s[n,m]
  out = w_out @ z_flat + b_out              (z chan = n*64+d)

Host-side exact refactor (fp64 weight folding):
  KW[(n,m), c] = sum_k key_p[n,k,m] w_in[k*8+n, c]   -> stage A: A = KW @ x
  kb[(n,m)]    = sum_k key_p[n,k,m] b_in[k*8+n]
  WM[o, (n,m)] = (sum_d w_out[o, n*64+d] memory[n,m,d]) * exp(kb[(n,m)])
  softmax(A + kb) folded:  E = exp(A);  wsum[n] = sum_m exp(kb) E;  S^ = E / wsum
  out = WM @ S^ + b_out

On-chip (per core = one batch, pixels chunked by 512):
  stage A: 2 K-tile matmuls -> PSUM [128 nm, 512 pix]  (4 nm tiles)
  exp:     ScalarE activation PSUM->SBUF
  wsum:    matmul with block-diagonal [128,128] (ekb-weighted head-indicator)
           -> per-head sums already broadcast across the 128 partitions
  recip:   VectorE PSUM->SBUF;  S^ = E * R  (VectorE)
  stage B: 8 accumulating matmuls -> PSUM [128 out, 512 pix] (2 o tiles x 4 K)
  bias:    VectorE tensor_scalar_add, DMA out.
"""

import os
import sys

import numpy as np

for _p in ("/opt/trn_rl_repo", "/root/.axon_site/_ro/trn_rl_repo"):
    if os.path.isdir(_p) and _p not in sys.path:
        sys.path.insert(0, _p)

import concourse.bass as bass  # noqa: E402
import concourse.tile as tile  # noqa: E402
from concourse import bacc, bass_utils, mybir  # noqa: E402

N_CORES = 8
C_IN = 256
NM = 512  # heads * mem_dim, channel order (n outer, m inner)
C_OUT = 256
NPIX = 64 * 64
CHUNK = 512
N_CHUNKS = NPIX // CHUNK
FP32 = mybir.dt.float32

_CACHED_NC = None


def _build_nc():
    nc = bacc.Bacc(
        "TRN2",
        target_bir_lowering=False,
        debug=False,
        enable_asserts=True,
        num_devices=N_CORES,
    )
    x_d = nc.dram_tensor("x", [C_IN, NPIX], FP32, kind="ExternalInput")
    kwt_d = nc.dram_tensor("kwt", [C_IN, NM], FP32, kind="ExternalInput")
    sumw_d = nc.dram_tensor("sumw", [128, NM], FP32, kind="ExternalInput")
    wmt_d = nc.dram_tensor("wmt", [NM, C_OUT], FP32, kind="ExternalInput")
    bout_d = nc.dram_tensor("bout", [128, 2], FP32, kind="ExternalInput")
    out_d = nc.dram_tensor("out", [C_OUT, NPIX], FP32, kind="ExternalOutput")

    Exp = mybir.ActivationFunctionType.Exp

    with tile.TileContext(nc) as tc:
        with (
            tc.tile_pool(name="wpool", bufs=1) as wpool,
            tc.tile_pool(name="xpool", bufs=3) as xpool,
            tc.tile_pool(name="epool", bufs=4) as epool,
            tc.tile_pool(name="rpool", bufs=3) as rpool,
            tc.tile_pool(name="spool", bufs=4) as spool,
            tc.tile_pool(name="opool", bufs=3) as opool,
            tc.tile_pool(name="pa", bufs=2, space="PSUM") as pa,
            tc.tile_pool(name="ps", bufs=2, space="PSUM") as ps,
            tc.tile_pool(name="po", bufs=2, space="PSUM") as po,
        ):
            kwt = []
            for i in range(2):
                t_ = wpool.tile([128, NM], FP32, name=f"kwt{i}", tag=f"kwt{i}")
                nc.sync.dma_start(t_[:], kwt_d[i * 128 : (i + 1) * 128, :])
                kwt.append(t_)
            sumw = wpool.tile([128, NM], FP32, name="sumw", tag="sumw")
            nc.sync.dma_start(sumw[:], sumw_d[:, :])
            wmt = []
            for t in range(4):
                t_ = wpool.tile([128, C_OUT], FP32, name=f"wmt{t}", tag=f"wmt{t}")
                nc.sync.dma_start(t_[:], wmt_d[t * 128 : (t + 1) * 128, :])
                wmt.append(t_)
            bout = wpool.tile([128, 2], FP32, name="bout", tag="bout")
            nc.sync.dma_start(bout[:], bout_d[:, :])

            for j in range(N_CHUNKS):
                xc = []
                for i in range(2):
                    t_ = xpool.tile([128, CHUNK], FP32, name=f"x{i}_{j}", tag=f"x{i}")
                    nc.sync.dma_start(
                        t_[:], x_d[i * 128 : (i + 1) * 128, j * CHUNK : (j + 1) * CHUNK]
                    )
                    xc.append(t_)

                s_tiles = []
                for t in range(4):
                    a_ps = pa.tile([128, CHUNK], FP32, name=f"pa_{j}_{t}", tag="pa")
                    for i in range(2):
                        nc.tensor.matmul(
                            a_ps[:],
                            kwt[i][:, t * 128 : (t + 1) * 128],
                            xc[i][:],
                            start=(i == 0),
                            stop=(i == 1),
                        )
                    e_sb = epool.tile([128, CHUNK], FP32, name=f"e_{j}_{t}", tag="e")
                    nc.scalar.activation(e_sb[:], a_ps[:], Exp)
                    s_ps = ps.tile([128, CHUNK], FP32, name=f"ps_{j}_{t}", tag="ps")
                    nc.tensor.matmul(
                        s_ps[:],
                        sumw[:, t * 128 : (t + 1) * 128],
                        e_sb[:],
                        start=True,
                        stop=True,
                    )
                    r_sb = rpool.tile([128, CHUNK], FP32, name=f"r_{j}_{t}", tag="r")
                    nc.vector.reciprocal(r_sb[:], s_ps[:])
                    s_sb = spool.tile([128, CHUNK], FP32, name=f"s_{j}_{t}", tag="s")
                    nc.vector.tensor_mul(s_sb[:], e_sb[:], r_sb[:])
                    s_tiles.append(s_sb)

                po_t = [po.tile([128, CHUNK], FP32, name=f"po{o}_{j}", tag=f"po{o}") for o in range(2)]
                for t in range(4):
                    for o in range(2):
                        nc.tensor.matmul(
                            po_t[o][:],
                            wmt[t][:, o * 128 : (o + 1) * 128],
                            s_tiles[t][:],
                            start=(t == 0),
                            stop=(t == 3),
                        )
                for o in range(2):
                    o_sb = opool.tile([128, CHUNK], FP32, name=f"o{o}_{j}", tag=f"o{o}")
                    nc.vector.tensor_scalar_add(o_sb[:], po_t[o][:], bout[:, o : o + 1])
                    nc.sync.dma_start(
                        out_d[o * 128 : (o + 1) * 128, j * CHUNK : (j + 1) * CHUNK],
                        o_sb[:],
                    )

    nc.compile()
    return nc


def _fold_weights(key_p, memory, w_in, b_in, w_out, b_out):
    key_p = np.asarray(key_p, np.float64)
    memory = np.asarray(memory, np.float64)
    w_in = np.asarray(w_in, np.float64)
    b_in = np.asarray(b_in, np.float64)
    w_out = np.asarray(w_out, np.float64)
    b_out = np.asarray(b_out, np.float64)

    w_in_r = w_in.reshape(64, 8, C_IN)  # [k, n, c]
    kw = np.einsum("nkm,knc->nmc", key_p, w_in_r)  # [n, m, c]
    kwt = np.ascontiguousarray(kw.reshape(NM, C_IN).T.astype(np.float32))

    kb = np.einsum("nkm,kn->nm", key_p, b_in.reshape(64, 8))  # [n, m]
    ekb = np.exp(kb).reshape(NM)  # (n,m) flat

    w_out_r = w_out.reshape(C_OUT, 8, 64)  # [o, n, d]
    wm = np.einsum("ond,nmd->onm", w_out_r, memory)  # [o, n, m]
    wmp = wm.reshape(C_OUT, NM) * ekb[None, :]
    wmt = np.ascontiguousarray(wmp.T.astype(np.float32))

    sumw = np.zeros((128, NM), np.float32)
    for t in range(4):
        ekb_t = ekb[128 * t : 128 * (t + 1)]
        blk = np.zeros((128, 128))
        blk[:64, :64] = ekb_t[:64, None]
        blk[64:, 64:] = ekb_t[64:, None]
        sumw[:, 128 * t : 128 * (t + 1)] = blk

    bout = np.ascontiguousarray(b_out.reshape(2, 128).T.astype(np.float32))
    return kwt, sumw, wmt, bout


def kernel_with_results(trace=False, tmpdir=None, **inputs):
    global _CACHED_NC
    x = np.asarray(inputs["x"], np.float32)  # [8, 256, 64, 64]
    kwt, sumw, wmt, bout = _fold_weights(
        inputs["key_p"],
        inputs["memory"],
        inputs["w_in"],
        inputs["b_in"],
        inputs["w_out"],
        inputs["b_out"],
    )
    if _CACHED_NC is None:
        _CACHED_NC = _build_nc()
    nc = _CACHED_NC

    in_maps = [
        {
            "x": np.ascontiguousarray(x[b].reshape(C_IN, NPIX)),
            "kwt": kwt,
            "sumw": sumw,
            "wmt": wmt,
            "bout": bout,
        }
        for b in range(N_CORES)
    ]
    res = bass_utils.run_bass_kernel_spmd(
        nc, in_maps, core_ids=list(range(N_CORES)), trace=trace, tmpdir=tmpdir
    )
    out = np.stack(
        [res.results[b]["out"].reshape(C_OUT, 64, 64) for b in range(N_CORES)]
    ).astype(np.float32)
    return out, res


def kernel(**inputs):
    out, _ = kernel_with_results(trace=False, **inputs)
    return out


# revision 5
# speedup vs baseline: 3.7234x; 3.7234x over previous
"""Trainium2 Bass kernel for KeyChannelwiseMemoryMultiHead.

Math: for each pixel vector x (256):
  y1 = w_in @ x + b_in                      (512 = 64 key x 8 heads, chan = k*8+n)
  a[n,m] = sum_k key_p[n,k,m] * y1[k*8+n]   (per-head key matmul)
  s = softmax_m(a[n,:])
  z[n,d] = sum_m memory[n,m,d] * s[n,m]
  out = w_out @ z_flat + b_out              (z chan = n*64+d)

Host-side exact refactor (fp64 weight folding):
  KW[(n,m), c] = sum_k key_p[n,k,m] w_in[k*8+n, c]   -> stage A: A = KW @ x
  kb[(n,m)]    = sum_k key_p[n,k,m] b_in[k*8+n]
  WM[o, (n,m)] = (sum_d w_out[o, n*64+d] memory[n,m,d]) * exp(kb[(n,m)])
  softmax(A + kb) folded:  E = exp(A);  wsum[n] = sum_m exp(kb) E;  S^ = E / wsum
  out = WM @ S^ + b_out

On-chip (per core = one batch, pixels chunked by 512):
  stage A: 2 K-tile matmuls -> PSUM [128 nm, 512 pix]  (4 nm tiles)
  exp:     ScalarE activation PSUM->SBUF
  wsum:    matmul with block-diagonal [128,128] (ekb-weighted head-indicator)
           -> per-head sums already broadcast across the 128 partitions
  recip:   VectorE PSUM->SBUF;  S^ = E * R  (VectorE)
  stage B: 8 accumulating matmuls -> PSUM [128 out, 512 pix] (2 o tiles x 4 K)
  bias:    VectorE tensor_scalar_add, DMA out.
"""

import os
import sys

import numpy as np

for _p in ("/opt/trn_rl_repo", "/root/.axon_site/_ro/trn_rl_repo"):
    if os.path.isdir(_p) and _p not in sys.path:
        sys.path.insert(0, _p)

import concourse.bass as bass  # noqa: E402
import concourse.tile as tile  # noqa: E402
from concourse import bacc, bass_utils, mybir  # noqa: E402
from concourse import dve_ops as _dve_ops  # noqa: E402
from concourse.dve_spec import (  # noqa: E402
    AluOp,
    Bin,
    C0,
    C1,
    Spec,
    Src0,
    Src1,
    _has_src1,
    lower,
)
from concourse.dve_table_gen import dve_ver_for  # noqa: E402
from concourse.dve_uop import DveOpSpec  # noqa: E402

N_CORES = 8
C_IN = 256
NM = 512  # heads * mem_dim, channel order (n outer, m inner)
C_OUT = 256
NPIX = 64 * 64
CHUNK = 512
N_CHUNKS = NPIX // CHUNK
FP32 = mybir.dt.float32
FP32R = mybir.dt.float32r
# Chebyshev seed constants shared with RECIPROCAL_APPROX_FAST; after ONE
# Newton step the recip rel-err is balanced at ~1.7e-3 (minimax pair).
_RC0 = -0.23549792
_RC1 = 2.0017324

_FUSED_OP = None


def _register_fused_divmul():
    """out = in1 * approx_recip(in0): BITWISE_NOT exponent-flip seed +
    one inline Newton pass + multiply by in1 -- single DVE pass replacing
    reciprocal()+tensor_mul() on the softmax normalization path."""
    global _FUSED_OP
    if _FUSED_OP is not None:
        return _FUSED_OP
    name = "RECIP1NR_MUL_ANT"
    _not_x = Bin(AluOp.BITWISE_NOT, Src0, Src0)
    _y0 = _not_x * C0
    _y1 = _y0 * (C1 - Src0 * _y0)

    def _ref(in0, in1, c0, c1, c2):
        not_x = (~in0.view(np.int32)).view(np.float32)
        y0 = not_x * c0
        y1 = y0 * (c1 - in0 * y0)
        return y1 * in1

    spec = Spec(body=_y1 * Src1, reference=_ref)
    row = max(_dve_ops._SUB_OPCODE_FOR_NAME.values()) + 1
    assert row < 0x20
    _dve_ops._SUB_OPCODE_FOR_NAME[name] = row
    shas = {}
    for ver in ("v3",):
        s = DveOpSpec(name=name, opcode=row, uops=lower(spec, ver=ver),
                      rd1_en=_has_src1(spec))
        shas[ver] = s.sha(ver)
    op = _dve_ops.DveOp(name, spec, subdim=False, uops_sha=shas)
    _dve_ops.OPS.append(op)
    _dve_ops.CUSTOM_DVE_SPECS[name] = spec
    _FUSED_OP = op
    return op

_CACHED_NC = None


def _build_nc():
    nc = bacc.Bacc(
        "TRN2",
        target_bir_lowering=False,
        debug=False,
        enable_asserts=True,
        num_devices=N_CORES,
    )
    x_d = nc.dram_tensor("x", [C_IN, NPIX], FP32R, kind="ExternalInput")
    kwt_d = nc.dram_tensor("kwt", [C_IN, NM], FP32R, kind="ExternalInput")
    sumw_d = nc.dram_tensor("sumw", [128, NM], FP32R, kind="ExternalInput")
    wmt_d = nc.dram_tensor("wmt", [NM, C_OUT], FP32R, kind="ExternalInput")
    bout_d = nc.dram_tensor("bout", [128, 2], FP32, kind="ExternalInput")
    out_d = nc.dram_tensor("out", [C_OUT, NPIX], FP32, kind="ExternalOutput")

    Exp = mybir.ActivationFunctionType.Exp
    fused = _register_fused_divmul()

    with tile.TileContext(nc) as tc:
        with (
            tc.tile_pool(name="wpool", bufs=1) as wpool,
            tc.tile_pool(name="xpool", bufs=3) as xpool,
            tc.tile_pool(name="epool", bufs=4) as epool,
            tc.tile_pool(name="rpool", bufs=3) as rpool,
            tc.tile_pool(name="spool", bufs=4) as spool,
            tc.tile_pool(name="opool", bufs=3) as opool,
            tc.tile_pool(name="pa", bufs=2, space="PSUM") as pa,
            tc.tile_pool(name="ps", bufs=2, space="PSUM") as ps,
            tc.tile_pool(name="po", bufs=2, space="PSUM") as po,
        ):
            kwt = []
            for i in range(2):
                t_ = wpool.tile([128, NM], FP32R, name=f"kwt{i}", tag=f"kwt{i}")
                nc.sync.dma_start(t_[:], kwt_d[i * 128 : (i + 1) * 128, :])
                kwt.append(t_)
            sumw = wpool.tile([128, NM], FP32R, name="sumw", tag="sumw")
            nc.sync.dma_start(sumw[:], sumw_d[:, :])
            wmt = []
            for t in range(4):
                t_ = wpool.tile([128, C_OUT], FP32R, name=f"wmt{t}", tag=f"wmt{t}")
                nc.sync.dma_start(t_[:], wmt_d[t * 128 : (t + 1) * 128, :])
                wmt.append(t_)
            bout = wpool.tile([128, 2], FP32, name="bout", tag="bout")
            nc.sync.dma_start(bout[:], bout_d[:, :])

            for j in range(N_CHUNKS):
                xc = []
                for i in range(2):
                    t_ = xpool.tile([128, CHUNK], FP32R, name=f"x{i}_{j}", tag=f"x{i}")
                    nc.sync.dma_start(
                        t_[:], x_d[i * 128 : (i + 1) * 128, j * CHUNK : (j + 1) * CHUNK]
                    )
                    xc.append(t_)

                s_tiles = []
                for t in range(4):
                    a_ps = pa.tile([128, CHUNK], FP32, name=f"pa_{j}_{t}", tag="pa")
                    for i in range(2):
                        nc.tensor.matmul(
                            a_ps[:],
                            kwt[i][:, t * 128 : (t + 1) * 128],
                            xc[i][:],
                            start=(i == 0),
                            stop=(i == 1),
                        )
                    e_sb = epool.tile([128, CHUNK], FP32R, name=f"e_{j}_{t}", tag="e")
                    nc.scalar.activation(e_sb[:], a_ps[:], Exp)
                    s_ps = ps.tile([128, CHUNK], FP32, name=f"ps_{j}_{t}", tag="ps")
                    nc.tensor.matmul(
                        s_ps[:],
                        sumw[:, t * 128 : (t + 1) * 128],
                        e_sb[:],
                        start=True,
                        stop=True,
                    )
                    s_sb = spool.tile([128, CHUNK], FP32R, name=f"s_{j}_{t}", tag="s")
                    nc.vector._custom_dve(
                        fused,
                        out=s_sb[:],
                        in0=s_ps[:],
                        in1=e_sb[:],
                        s0=_RC0,
                        s1=_RC1,
                    )
                    s_tiles.append(s_sb)

                po_t = [po.tile([128, CHUNK], FP32, name=f"po{o}_{j}", tag=f"po{o}") for o in range(2)]
                for t in range(4):
                    for o in range(2):
                        nc.tensor.matmul(
                            po_t[o][:],
                            wmt[t][:, o * 128 : (o + 1) * 128],
                            s_tiles[t][:],
                            start=(t == 0),
                            stop=(t == 3),
                        )
                for o in range(2):
                    o_sb = opool.tile([128, CHUNK], FP32, name=f"o{o}_{j}", tag=f"o{o}")
                    if o == 0:
                        nc.scalar.activation(
                            o_sb[:],
                            po_t[o][:],
                            mybir.ActivationFunctionType.Identity,
                            bias=bout[:, o : o + 1],
                        )
                    else:
                        nc.vector.tensor_scalar_add(
                            o_sb[:], po_t[o][:], bout[:, o : o + 1]
                        )
                    nc.sync.dma_start(
                        out_d[o * 128 : (o + 1) * 128, j * CHUNK : (j + 1) * CHUNK],
                        o_sb[:],
                    )

    nc.compile()
    return nc


def _fold_weights(key_p, memory, w_in, b_in, w_out, b_out):
    key_p = np.asarray(key_p, np.float64)
    memory = np.asarray(memory, np.float64)
    w_in = np.asarray(w_in, np.float64)
    b_in = np.asarray(b_in, np.float64)
    w_out = np.asarray(w_out, np.float64)
    b_out = np.asarray(b_out, np.float64)

    w_in_r = w_in.reshape(64, 8, C_IN)  # [k, n, c]
    kw = np.einsum("nkm,knc->nmc", key_p, w_in_r)  # [n, m, c]
    kwt = np.ascontiguousarray(kw.reshape(NM, C_IN).T.astype(np.float32))

    kb = np.einsum("nkm,kn->nm", key_p, b_in.reshape(64, 8))  # [n, m]
    ekb = np.exp(kb).reshape(NM)  # (n,m) flat

    w_out_r = w_out.reshape(C_OUT, 8, 64)  # [o, n, d]
    wm = np.einsum("ond,nmd->onm", w_out_r, memory)  # [o, n, m]
    wmp = wm.reshape(C_OUT, NM) * ekb[None, :]
    wmt = np.ascontiguousarray(wmp.T.astype(np.float32))

    sumw = np.zeros((128, NM), np.float32)
    for t in range(4):
        ekb_t = ekb[128 * t : 128 * (t + 1)]
        blk = np.zeros((128, 128))
        blk[:64, :64] = ekb_t[:64, None]
        blk[64:, 64:] = ekb_t[64:, None]
        sumw[:, 128 * t : 128 * (t + 1)] = blk

    bout = np.ascontiguousarray(b_out.reshape(2, 128).T.astype(np.float32))
    return kwt, sumw, wmt, bout


def kernel_with_results(trace=False, tmpdir=None, **inputs):
    global _CACHED_NC
    x = np.asarray(inputs["x"], np.float32)  # [8, 256, 64, 64]
    kwt, sumw, wmt, bout = _fold_weights(
        inputs["key_p"],
        inputs["memory"],
        inputs["w_in"],
        inputs["b_in"],
        inputs["w_out"],
        inputs["b_out"],
    )
    if _CACHED_NC is None:
        _CACHED_NC = _build_nc()
    nc = _CACHED_NC

    in_maps = [
        {
            "x": np.ascontiguousarray(x[b].reshape(C_IN, NPIX)),
            "kwt": kwt,
            "sumw": sumw,
            "wmt": wmt,
            "bout": bout,
        }
        for b in range(N_CORES)
    ]
    res = bass_utils.run_bass_kernel_spmd(
        nc, in_maps, core_ids=list(range(N_CORES)), trace=trace, tmpdir=tmpdir
    )
    out = np.stack(
        [res.results[b]["out"].reshape(C_OUT, 64, 64) for b in range(N_CORES)]
    ).astype(np.float32)
    return out, res


def kernel(**inputs):
    out, _ = kernel_with_results(trace=False, **inputs)
    return out


# revision 6
# speedup vs baseline: 3.9719x; 1.0667x over previous
"""Trainium2 Bass kernel for KeyChannelwiseMemoryMultiHead.

Math: for each pixel vector x (256):
  y1 = w_in @ x + b_in                      (512 = 64 key x 8 heads, chan = k*8+n)
  a[n,m] = sum_k key_p[n,k,m] * y1[k*8+n]   (per-head key matmul)
  s = softmax_m(a[n,:])
  z[n,d] = sum_m memory[n,m,d] * s[n,m]
  out = w_out @ z_flat + b_out              (z chan = n*64+d)

Host-side exact refactor (fp64 weight folding):
  KW[(n,m), c] = sum_k key_p[n,k,m] w_in[k*8+n, c]   -> stage A: A = KW @ x
  kb[(n,m)]    = sum_k key_p[n,k,m] b_in[k*8+n]
  WM[o, (n,m)] = (sum_d w_out[o, n*64+d] memory[n,m,d]) * exp(kb[(n,m)])
  softmax(A + kb) folded:  E = exp(A);  wsum[n] = sum_m exp(kb) E;  S^ = E / wsum
  out = WM @ S^ + b_out

On-chip (per core = one batch, pixels chunked by 512):
  stage A: 2 K-tile matmuls -> PSUM [128 nm, 512 pix]  (4 nm tiles)
  exp:     ScalarE activation PSUM->SBUF
  wsum:    matmul with block-diagonal [128,128] (ekb-weighted head-indicator)
           -> per-head sums already broadcast across the 128 partitions
  recip:   VectorE PSUM->SBUF;  S^ = E * R  (VectorE)
  stage B: 8 accumulating matmuls -> PSUM [128 out, 512 pix] (2 o tiles x 4 K)
  bias:    VectorE tensor_scalar_add, DMA out.
"""

import os
import sys

import numpy as np

for _p in ("/opt/trn_rl_repo", "/root/.axon_site/_ro/trn_rl_repo"):
    if os.path.isdir(_p) and _p not in sys.path:
        sys.path.insert(0, _p)

import concourse.bass as bass  # noqa: E402
import concourse.tile as tile  # noqa: E402
from concourse import bacc, bass_utils, mybir  # noqa: E402
from concourse import dve_ops as _dve_ops  # noqa: E402
from concourse.dve_spec import (  # noqa: E402
    AluOp,
    Bin,
    C0,
    C1,
    Spec,
    Src0,
    Src1,
    _has_src1,
    lower,
)
from concourse.dve_table_gen import dve_ver_for  # noqa: E402
from concourse.dve_uop import DveOpSpec  # noqa: E402

N_CORES = 8
C_IN = 256
NM = 512  # heads * mem_dim, channel order (n outer, m inner)
C_OUT = 256
NPIX = 64 * 64
CHUNK = 512
N_CHUNKS = NPIX // CHUNK
FP32 = mybir.dt.float32
FP32R = mybir.dt.float32r
BF16 = mybir.dt.bfloat16
# Chebyshev seed constants shared with RECIPROCAL_APPROX_FAST; after ONE
# Newton step the recip rel-err is balanced at ~1.7e-3 (minimax pair).
_RC0 = -0.23549792
_RC1 = 2.0017324

_FUSED_OP = None


def _register_fused_divmul():
    """out = in1 * approx_recip(in0): BITWISE_NOT exponent-flip seed +
    one inline Newton pass + multiply by in1 -- single DVE pass replacing
    reciprocal()+tensor_mul() on the softmax normalization path."""
    global _FUSED_OP
    if _FUSED_OP is not None:
        return _FUSED_OP
    name = "RECIP1NR_MUL_ANT"
    _not_x = Bin(AluOp.BITWISE_NOT, Src0, Src0)
    _y0 = _not_x * C0
    _y1 = _y0 * (C1 - Src0 * _y0)

    def _ref(in0, in1, c0, c1, c2):
        not_x = (~in0.view(np.int32)).view(np.float32)
        y0 = not_x * c0
        y1 = y0 * (c1 - in0 * y0)
        return y1 * in1

    spec = Spec(body=_y1 * Src1, reference=_ref)
    row = max(_dve_ops._SUB_OPCODE_FOR_NAME.values()) + 1
    assert row < 0x20
    _dve_ops._SUB_OPCODE_FOR_NAME[name] = row
    shas = {}
    for ver in ("v3",):
        s = DveOpSpec(name=name, opcode=row, uops=lower(spec, ver=ver),
                      rd1_en=_has_src1(spec))
        shas[ver] = s.sha(ver)
    op = _dve_ops.DveOp(name, spec, subdim=False, uops_sha=shas)
    _dve_ops.OPS.append(op)
    _dve_ops.CUSTOM_DVE_SPECS[name] = spec
    _FUSED_OP = op
    return op

_CACHED_NC = None


def _build_nc():
    nc = bacc.Bacc(
        "TRN2",
        target_bir_lowering=False,
        debug=False,
        enable_asserts=True,
        num_devices=N_CORES,
    )
    x_d = nc.dram_tensor("x", [C_IN, NPIX], BF16, kind="ExternalInput")
    kwt_d = nc.dram_tensor("kwt", [C_IN, NM], BF16, kind="ExternalInput")
    sumw_d = nc.dram_tensor("sumw", [128, NM], BF16, kind="ExternalInput")
    wmt_d = nc.dram_tensor("wmt", [NM, C_OUT], BF16, kind="ExternalInput")
    bout_d = nc.dram_tensor("bout", [128, 2], FP32, kind="ExternalInput")
    out_d = nc.dram_tensor("out", [C_OUT, NPIX], FP32, kind="ExternalOutput")

    Exp = mybir.ActivationFunctionType.Exp
    fused = _register_fused_divmul()

    with tile.TileContext(nc) as tc:
        with (
            tc.tile_pool(name="wpool", bufs=1) as wpool,
            tc.tile_pool(name="xpool", bufs=3) as xpool,
            tc.tile_pool(name="epool", bufs=4) as epool,
            tc.tile_pool(name="rpool", bufs=3) as rpool,
            tc.tile_pool(name="spool", bufs=4) as spool,
            tc.tile_pool(name="opool", bufs=3) as opool,
            tc.tile_pool(name="pa", bufs=2, space="PSUM") as pa,
            tc.tile_pool(name="ps", bufs=2, space="PSUM") as ps,
            tc.tile_pool(name="po", bufs=2, space="PSUM") as po,
        ):
            kwt = []
            for i in range(2):
                t_ = wpool.tile([128, NM], BF16, name=f"kwt{i}", tag=f"kwt{i}")
                nc.sync.dma_start(t_[:], kwt_d[i * 128 : (i + 1) * 128, :])
                kwt.append(t_)
            sumw = wpool.tile([128, NM], BF16, name="sumw", tag="sumw")
            nc.sync.dma_start(sumw[:], sumw_d[:, :])
            wmt = []
            for t in range(4):
                t_ = wpool.tile([128, C_OUT], BF16, name=f"wmt{t}", tag=f"wmt{t}")
                nc.sync.dma_start(t_[:], wmt_d[t * 128 : (t + 1) * 128, :])
                wmt.append(t_)
            bout = wpool.tile([128, 2], FP32, name="bout", tag="bout")
            nc.sync.dma_start(bout[:], bout_d[:, :])

            for j in range(N_CHUNKS):
                xc = []
                for i in range(2):
                    t_ = xpool.tile([128, CHUNK], BF16, name=f"x{i}_{j}", tag=f"x{i}")
                    nc.sync.dma_start(
                        t_[:], x_d[i * 128 : (i + 1) * 128, j * CHUNK : (j + 1) * CHUNK]
                    )
                    xc.append(t_)

                s_tiles = []
                for t in range(4):
                    a_ps = pa.tile([128, CHUNK], FP32, name=f"pa_{j}_{t}", tag="pa")
                    for i in range(2):
                        nc.tensor.matmul(
                            a_ps[:],
                            kwt[i][:, t * 128 : (t + 1) * 128],
                            xc[i][:],
                            start=(i == 0),
                            stop=(i == 1),
                        )
                    e_sb = epool.tile([128, CHUNK], BF16, name=f"e_{j}_{t}", tag="e")
                    nc.scalar.activation(e_sb[:], a_ps[:], Exp)
                    s_ps = ps.tile([128, CHUNK], FP32, name=f"ps_{j}_{t}", tag="ps")
                    nc.tensor.matmul(
                        s_ps[:],
                        sumw[:, t * 128 : (t + 1) * 128],
                        e_sb[:],
                        start=True,
                        stop=True,
                    )
                    s_sb = spool.tile([128, CHUNK], BF16, name=f"s_{j}_{t}", tag="s")
                    nc.vector._custom_dve(
                        fused,
                        out=s_sb[:],
                        in0=s_ps[:],
                        in1=e_sb[:],
                        s0=_RC0,
                        s1=_RC1,
                    )
                    s_tiles.append(s_sb)

                po_t = [po.tile([128, CHUNK], FP32, name=f"po{o}_{j}", tag=f"po{o}") for o in range(2)]
                for t in range(4):
                    for o in range(2):
                        nc.tensor.matmul(
                            po_t[o][:],
                            wmt[t][:, o * 128 : (o + 1) * 128],
                            s_tiles[t][:],
                            start=(t == 0),
                            stop=(t == 3),
                        )
                for o in range(2):
                    o_sb = opool.tile([128, CHUNK], FP32, name=f"o{o}_{j}", tag=f"o{o}")
                    if o == 0:
                        nc.scalar.activation(
                            o_sb[:],
                            po_t[o][:],
                            mybir.ActivationFunctionType.Identity,
                            bias=bout[:, o : o + 1],
                        )
                    else:
                        nc.vector.tensor_scalar_add(
                            o_sb[:], po_t[o][:], bout[:, o : o + 1]
                        )
                    nc.sync.dma_start(
                        out_d[o * 128 : (o + 1) * 128, j * CHUNK : (j + 1) * CHUNK],
                        o_sb[:],
                    )

    nc.compile()
    return nc


def _fold_weights(key_p, memory, w_in, b_in, w_out, b_out):
    key_p = np.asarray(key_p, np.float64)
    memory = np.asarray(memory, np.float64)
    w_in = np.asarray(w_in, np.float64)
    b_in = np.asarray(b_in, np.float64)
    w_out = np.asarray(w_out, np.float64)
    b_out = np.asarray(b_out, np.float64)

    w_in_r = w_in.reshape(64, 8, C_IN)  # [k, n, c]
    kw = np.einsum("nkm,knc->nmc", key_p, w_in_r)  # [n, m, c]
    import ml_dtypes

    kwt = np.ascontiguousarray(kw.reshape(NM, C_IN).T.astype(ml_dtypes.bfloat16))

    kb = np.einsum("nkm,kn->nm", key_p, b_in.reshape(64, 8))  # [n, m]
    ekb = np.exp(kb).reshape(NM)  # (n,m) flat

    w_out_r = w_out.reshape(C_OUT, 8, 64)  # [o, n, d]
    wm = np.einsum("ond,nmd->onm", w_out_r, memory)  # [o, n, m]
    wmp = wm.reshape(C_OUT, NM) * ekb[None, :]
    wmt = np.ascontiguousarray(wmp.T.astype(ml_dtypes.bfloat16))

    sumw = np.zeros((128, NM), ml_dtypes.bfloat16)
    for t in range(4):
        ekb_t = ekb[128 * t : 128 * (t + 1)]
        blk = np.zeros((128, 128))
        blk[:64, :64] = ekb_t[:64, None]
        blk[64:, 64:] = ekb_t[64:, None]
        sumw[:, 128 * t : 128 * (t + 1)] = blk

    bout = np.ascontiguousarray(b_out.reshape(2, 128).T.astype(np.float32))
    return kwt, sumw, wmt, bout


import ml_dtypes as _mld

_ml_bf16 = _mld.bfloat16


def kernel_with_results(trace=False, tmpdir=None, **inputs):
    global _CACHED_NC
    x = np.asarray(inputs["x"], np.float32)  # [8, 256, 64, 64]
    kwt, sumw, wmt, bout = _fold_weights(
        inputs["key_p"],
        inputs["memory"],
        inputs["w_in"],
        inputs["b_in"],
        inputs["w_out"],
        inputs["b_out"],
    )
    if _CACHED_NC is None:
        _CACHED_NC = _build_nc()
    nc = _CACHED_NC

    in_maps = [
        {
            "x": np.ascontiguousarray(
                x[b].reshape(C_IN, NPIX).astype(_ml_bf16)
            ),
            "kwt": kwt,
            "sumw": sumw,
            "wmt": wmt,
            "bout": bout,
        }
        for b in range(N_CORES)
    ]
    res = bass_utils.run_bass_kernel_spmd(
        nc, in_maps, core_ids=list(range(N_CORES)), trace=trace, tmpdir=tmpdir
    )
    out = np.stack(
        [res.results[b]["out"].reshape(C_OUT, 64, 64) for b in range(N_CORES)]
    ).astype(np.float32)
    return out, res


def kernel(**inputs):
    out, _ = kernel_with_results(trace=False, **inputs)
    return out
